# revision 25
# baseline (speedup 1.0000x reference)
"""InfoNCE lower-bound kernel for 8 Trainium2 NeuronCores (v3).

Math (reference):
  hx = x @ W1x.T ; hy = y @ W1y.T            [N, H]
  z_ij = relu(hx[j] + hy[i] + b1) . w2       (logit WITHOUT b2)
  T1[i,j] = softplus(z_ij + b2)
  T0[i]   = T1[i,i]
  lse[i]  = log(sum_j exp(T1[i,j])) = log(N + sum_j exp(z_ij + b2))
  out     = mean(T0) - (mean(lse) - log N)

Sharding: data-parallel over i (rows of the pair grid); each core gets 64
rows of y, x and params replicated.

v3 design (on top of the v2 fp32->bf16 baseline):
  * All inputs pre-formatted on the host into exact SBUF layouts so every
    input DMA is a flat [128, F] contiguous copy; issues are spread across
    both HWDGE engines (sync + scalar). Each DMA issue costs ~600ns of
    sequencer time regardless of size, so transfers are consolidated.
  * PE p-state warmup: a run of dummy [128,64] matmuls burns the slow
    p-state window while input DMAs are in flight.
  * z accumulation uses TWO psum banks (M=32 weight windows): row i ->
    bank i//32, psum row q=i%32. Every z matmul writes its bank's full
    [32,512] region (w2 chunk at col 31-q..+32 of a zero strip -> local
    col q). All accumulation into a bank is single-strip sequential (the
    cross-strip concurrent case corrupts on HW). Bank 0 completes after
    block 3, so its exp runs DURING blocks 4-7; only bank 1's exp is in
    the tail.
  * exp uses the Act accumulator (accum_out) to fuse the free-axis
    sum_j exp(z+b2) into the activation op - no DVE reduces at all.
  * The relu grid is split DVE 14 / Act 6 ops per 8-row block. Act rows
    ({2,5} rel.) write fp8 PAIR tiles [128,1024] = [t0|t1] consumed by ONE
    DoubleRow fp8 matmul each (2 k-tiles per stream at 0.5 cyc/row); Act
    also covers the t2 tail pairs 0,1 as fp8 pair-of-blocks tiles (one
    DoubleRow per 2 blocks). DVE tiles stay bf16 (4x DVE mode) with plain
    bf16 matmuls. fp8 noise is ~1e-4 abs on the result vs the 7.8e-4 abs
    error budget.
  * Device ships per-row partial results (sum_j exp(z+b2) and diag
    logits); the final ln/softplus/means run on the host.
"""

import math

import numpy as np

N = 512
XD = 768
YD = 768
H = 300
NCORES = 8
ISH = N // NCORES   # 64 rows per core
KD = XD // 128      # 6 contraction tiles of 128
HT = 3              # h tiles: 128, 128, 44
HSZ = [128, 128, H - 256]

ACT_I = (2, 5)                # rel. rows per block on Act -> fp8 DoubleRow
IORD_DVE = (0, 1, 3, 4, 6, 7)

# w2a_sb (bf16) column offsets: diag cols 0:4, then M=64 weight-window
# strips (chunk at col 63; lhsT window [63-q : 127-q] -> local col q).
W_T0, W_T1, W_T2 = 4, 131, 258
W2A_W = 386
# w2f8_sb (fp8 DoubleRow): windows are illegal (LdWeights dual-fp8 needs
# 16B-aligned APs with slot stride %16==0), so per-q blocks [128, 2, 64]:
# 16 t0|t1 blocks then 8 t2 pair-of-blocks blocks.
W2F8_T2 = 16 * 128
W2F8_W = 24 * 128

_CACHE = {}
TRACE = False
LAST_RESULTS = None


def _build_module():
    import concourse.bacc as bacc
    import concourse.mybir as mybir
    from concourse.tile import TileContext

    f32 = mybir.dt.float32
    bf16 = mybir.dt.bfloat16
    f8 = mybir.dt.float8e4
    AF = mybir.ActivationFunctionType
    ALU = mybir.AluOpType
    DROW = mybir.MatmulPerfMode.DoubleRow

    nc = bacc.Bacc("TRN2", target_bir_lowering=False, debug=False)

    # Per-core inputs (SPMD: same shapes, different data for yt/xtd slices).
    xt_p = nc.dram_tensor("xt_p", [128, KD * N], bf16, kind="ExternalInput")
    w1x_p = nc.dram_tensor("w1x_p", [128, KD * H], bf16, kind="ExternalInput")
    w1y_p = nc.dram_tensor("w1y_p", [128, KD * H], bf16, kind="ExternalInput")
    aux_p = nc.dram_tensor("aux_p", [128, 2 * KD * ISH + W2A_W], bf16, kind="ExternalInput")
    bcons = nc.dram_tensor("bcons", [128, HT + 1], f32, kind="ExternalInput")
    w2f8 = nc.dram_tensor("w2f8", [128, W2F8_W], f8, kind="ExternalInput")
    b1row = nc.dram_tensor("b1row", [1, 3 * 128], bf16, kind="ExternalInput")
    outS = nc.dram_tensor("outS", [128, 1], f32, kind="ExternalOutput")  # accum image
    outD = nc.dram_tensor("outD", [1, ISH], f32, kind="ExternalOutput")  # diag logits

    AUX_YT, AUX_XTD, AUX_W2A = 0, KD * ISH, 2 * KD * ISH

    with TileContext(nc) as tc:
        cpool = tc.alloc_tile_pool(name="consts", bufs=1)
        rpool = tc.alloc_tile_pool(name="work", bufs=32)
        pp_pre = tc.alloc_tile_pool(name="pp_pre", bufs=1, space="PSUM")
        pp_z = tc.alloc_tile_pool(name="pp_z", bufs=1, space="PSUM")
        pp_d = tc.alloc_tile_pool(name="pp_d", bufs=1, space="PSUM")

        # ---- constant tiles ----
        xt_sb = cpool.tile([128, KD * N], bf16, tag="xt")
        w1x_sb = cpool.tile([128, KD * H], bf16, tag="w1x")
        w1y_sb = cpool.tile([128, KD * H], bf16, tag="w1y")
        aux_sb = cpool.tile([128, 2 * KD * ISH + W2A_W], bf16, tag="aux")
        bc_sb = cpool.tile([128, HT + 1], f32, tag="bc")
        w2f8_sb = cpool.tile([128, W2F8_W], f8, tag="w2f8")
        b1r_sb = cpool.tile([1, 3 * 128], bf16, tag="b1r")

        hxb0 = cpool.tile([128, N], bf16, tag="hxb0")    # relu-arg x part (+b1), t0
        hxb1 = cpool.tile([128, N], bf16, tag="hxb1")    # t1
        hxb2p = cpool.tile([128, N], bf16, tag="hxb2p")  # t2 packed (rows 0:44, 64:108)
        hyf_sb = cpool.tile([128, HT * ISH], f32, tag="hyf")   # hy per h-tile (f32)
        hy2p = cpool.tile([128, ISH // 2], f32, tag="hy2p")    # packed t2 pairs
        ones64 = cpool.tile([1, ISH], bf16, tag="ones64")
        escr = cpool.tile([128, N], bf16, tag="escr")          # exp out (unread)
        sexp1 = cpool.tile([128, 1], f32, tag="sexp1")
        dlog = cpool.tile([1, ISH], f32, tag="dlog")
        warm = cpool.tile([128, 64], bf16, tag="warm")

        def w2a(lo, hi):
            return aux_sb[:, AUX_W2A + lo:AUX_W2A + hi]

        # ---- input DMAs ----
        def half(eng, dst, src, h):
            w = dst.shape[1] // 2
            eng.dma_start(dst[:, h * w:(h + 1) * w], src[:, h * w:(h + 1) * w])
        half(nc.sync, xt_sb, xt_p, 0)
        half(nc.scalar, w1x_sb, w1x_p, 0)
        half(nc.sync, xt_sb, xt_p, 1)
        half(nc.scalar, w1x_sb, w1x_p, 1)
        nc.sync.dma_start(aux_sb[:], aux_p[:])
        nc.scalar.dma_start(w1y_sb[:], w1y_p[:])
        nc.sync.dma_start(bc_sb[:], bcons[:])
        nc.scalar.dma_start(w2f8_sb[:], w2f8[:])
        nc.sync.dma_start(b1r_sb[:], b1row[:])

        # ---- PE p-state warmup: burn the slow-clock window on dummy
        # matmuls (no input deps) while the DMAs land ----
        nc.gpsimd.memset(warm[:], 0.0)
        wps = pp_d.tile([128, 64], f32, tag="warm_ps")
        for _ in range(40):
            nc.tensor.matmul(wps[0:64, :], lhsT=warm[:, 0:64], rhs=warm[:],
                             start=True, stop=True, skip_group_check=True)

        nc.gpsimd.memset(ones64[:], 1.0)
        # zero the packed-t2 operand tiles before their writers fill the
        # live rows, so the pair matmul's zero-weight rows multiply finite
        # values (NaN * 0 = NaN).
        nc.gpsimd.memset(hxb2p[:], 0.0)
        nc.gpsimd.memset(hy2p[:], 0.0)

        # ---- preamble: hxb (x part, +b1) and hy interleaved per h-tile.
        # PSUM->SBUF copies go through Act (idle during the preamble) so the
        # DVE spends its time on the relu grid only. ----
        for t in range(HT):
            hs = HSZ[t]
            ps = pp_pre.tile([128, N], f32, tag="pre512", bufs=2)
            for k in range(KD):
                nc.tensor.matmul(
                    ps[0:hs, :],
                    lhsT=w1x_sb[:, k * H + 128 * t: k * H + 128 * t + hs],
                    rhs=xt_sb[:, k * N:(k + 1) * N],
                    start=(k == 0), stop=(k == KD - 1),
                )
            dst = [hxb0, hxb1, hxb2p][t]
            nc.scalar.activation(
                dst[0:hs, :], ps[0:hs, :], AF.Identity, bias=bc_sb[0:hs, t:t + 1]
            )
            psy = pp_pre.tile([128, ISH], f32, tag="pre64", bufs=1)
            for k in range(KD):
                nc.tensor.matmul(
                    psy[0:hs, :],
                    lhsT=w1y_sb[:, k * H + 128 * t: k * H + 128 * t + hs],
                    rhs=aux_sb[:, AUX_YT + k * ISH:AUX_YT + (k + 1) * ISH],
                    start=(k == 0), stop=(k == KD - 1),
                )
            nc.scalar.activation(
                hyf_sb[0:hs, t * ISH:(t + 1) * ISH], psy[0:hs, :], AF.Identity
            )
            if t == 2:
                # packed pair layout: col p <- (even col 2p at rows 0:44,
                # odd col 2p+1 at rows 64:108)
                evens = psy[0:hs, :].rearrange("p (a two) -> p two a", two=2)
                nc.scalar.activation(hy2p[0:hs, :], evens[:, 0, :], AF.Identity)
                nc.scalar.activation(hy2p[64:64 + hs, :], evens[:, 1, :], AF.Identity)
        nc.vector.tensor_copy(hxb2p[64:64 + HSZ[2], :], hxb2p[0:HSZ[2], :])

        # ---- main loop: 8 blocks of 8 rows into ONE psum bank (M=64) ----
        # Row i (local) -> psum row q = i. Every z matmul writes the full
        # [64, 512] region (zero weight cols elsewhere), so accumulation is
        # plain sequential single-strip at PE position (0,0) — DoubleRow
        # forbids column tiling, and this also leaves one exp in the tail.
        def relu_op(eng, out_ap, in_ap, col_f32):
            if eng == 'A':
                nc.scalar.activation(out_ap, in_ap, AF.Relu, bias=col_f32)
            else:
                nc.vector.tensor_scalar(out_ap, in_ap, col_f32, 0.0, ALU.add, ALU.max)

        zbk = pp_z.tile([128, N], f32, tag="zp")

        NB = ISH // 8
        rp2t = {}   # p -> fp8 pair-of-blocks t2 tile
        for b in range(NB):
            q0 = 8 * b                 # q of the block's first row
            # --- relu ops ---
            rt = {}    # (i_rel, t) -> bf16 tile (DVE rows)
            rf8 = {}   # i_rel -> fp8 [128, 1024] pair tile (Act rows)
            rps = {}   # p -> bf16 t2 pair tile (DVE pairs 2, 3)
            for i_rel in IORD_DVE:
                for t in range(2):
                    r = rpool.tile([128, N], bf16, tag="r", bufs=28)
                    relu_op(
                        'D', r[:], [hxb0, hxb1][t][:],
                        hyf_sb[:, t * ISH + 8 * b + i_rel: t * ISH + 8 * b + i_rel + 1],
                    )
                    rt[(i_rel, t)] = r
            for i_rel in ACT_I:
                rf = rpool.tile([128, 2 * N], f8, tag="rf8", bufs=6)
                for t in range(2):
                    relu_op(
                        'A', rf[:, t * N:(t + 1) * N], [hxb0, hxb1][t][:],
                        hyf_sb[:, t * ISH + 8 * b + i_rel: t * ISH + 8 * b + i_rel + 1],
                    )
                rf8[i_rel] = rf
            for p in (0, 1):           # Act fp8 t2, paired across (b, b+1)
                if b % 2 == 0:
                    rp2t[p] = rpool.tile([128, 2 * N], f8, tag="rp2", bufs=4,
                                         name=f"rp2_{b}_{p}")
                relu_op('A', rp2t[p][:, (b % 2) * N:(b % 2 + 1) * N], hxb2p[:],
                        hy2p[:, 4 * b + p:4 * b + p + 1])
            for p in (2, 3):           # DVE bf16 t2 pairs
                rp = rpool.tile([128, N], bf16, tag="rp", bufs=6)
                relu_op('D', rp[:], hxb2p[:], hy2p[:, 4 * b + p:4 * b + p + 1])
                rps[p] = rp

            # --- z matmuls (PE order) ---
            zo = zbk[0:ISH, :]

            def zmm(i_rel, t, start=False):
                q = q0 + i_rel
                base = [W_T0, W_T1][t]
                nc.tensor.matmul(
                    zo, lhsT=w2a(base + 63 - q, base + 127 - q),
                    rhs=rt[(i_rel, t)][:], start=start, stop=False,
                    skip_group_check=True,
                )

            def zdr(i_rel):
                j = 2 * b + (0 if i_rel == ACT_I[0] else 1)
                nc.tensor.matmul(
                    zo,
                    lhsT=w2f8_sb[:, 128 * j:128 * (j + 1)]
                    .rearrange("p (two m) -> p two m", two=2),
                    rhs=rf8[i_rel][:].rearrange("p (two f) -> p two f", two=2),
                    start=False, stop=False, perf_mode=DROW,
                    skip_group_check=True,
                )

            def zpair(p):
                q = q0 + 2 * p
                nc.tensor.matmul(
                    zo, lhsT=w2a(W_T2 + 63 - q, W_T2 + 127 - q),
                    rhs=rps[p][:], start=False, stop=False,
                    skip_group_check=True,
                )

            def zdr_t2(p, stop):
                j2 = (b - 1) + p               # blocks (b-1, b), pair p
                nc.tensor.matmul(
                    zo,
                    lhsT=w2f8_sb[:, W2F8_T2 + 128 * j2:W2F8_T2 + 128 * (j2 + 1)]
                    .rearrange("p (two m) -> p two m", two=2),
                    rhs=rp2t[p][:].rearrange("p (two f) -> p two f", two=2),
                    start=False, stop=stop, perf_mode=DROW,
                    skip_group_check=True,
                )

            for t in range(2):
                for i_rel in IORD_DVE:
                    zmm(i_rel, t, start=(b == 0 and t == 0 and i_rel == IORD_DVE[0]))
            for i_rel in ACT_I:
                zdr(i_rel)
            zpair(2)
            zpair(3)
            if b % 2 == 1:
                zdr_t2(0, stop=False)
                zdr_t2(1, stop=(b == NB - 1))

            if b == 2:
                # ---- diag: dlog[i] = w2 . relu(hxd_i + hy_i + b1), fed
                # straight from PSUM (b1 added via a rank-1 K=1 matmul);
                # at b==2 so the aux/b1row DMAs land after grid start ----
                dps = pp_d.tile([128, ISH], f32, tag="dps")
                for dt_ in range(HT):
                    hs = HSZ[dt_]
                    psd = pp_pre.tile([128, ISH], f32, tag="pre64", bufs=1)
                    for k in range(KD):
                        nc.tensor.matmul(
                            psd[0:hs, :],
                            lhsT=w1x_sb[:, k * H + 128 * dt_: k * H + 128 * dt_ + hs],
                            rhs=aux_sb[:, AUX_XTD + k * ISH:AUX_XTD + (k + 1) * ISH],
                            start=(k == 0), stop=False,
                        )
                    nc.tensor.matmul(
                        psd[0:hs, :],
                        lhsT=b1r_sb[0:1, 128 * dt_:128 * dt_ + hs],
                        rhs=ones64[0:1, :],
                        start=False, stop=True,
                    )
                    dsum = rpool.tile([128, ISH], bf16, tag="dsum", bufs=2)
                    nc.vector.tensor_tensor(
                        dsum[0:hs, :], psd[0:hs, :],
                        hyf_sb[0:hs, dt_ * ISH:(dt_ + 1) * ISH], op=ALU.add,
                    )
                    dr = rpool.tile([128, ISH], bf16, tag="dr", bufs=2)
                    nc.vector.tensor_scalar(
                        dr[0:hs, :], dsum[0:hs, :], 0.0, None, ALU.max
                    )
                    dcol = AUX_W2A + (dt_ if dt_ < 2 else 2)
                    nc.tensor.matmul(
                        dps[0:1, :],
                        lhsT=aux_sb[0:hs, dcol:dcol + 1],
                        rhs=dr[0:hs, :],
                        start=(dt_ == 0), stop=(dt_ == HT - 1),
                    )
                nc.scalar.activation(dlog[0:1, :], dps[0:1, :], AF.Identity)
                nc.sync.dma_start(outD[0:1, :], dlog[0:1, :])

        # tail: one exp(z + b2) with fused free-axis accumulate sum_j over
        # the whole bank (rows 0:64 valid, the rest junk the host drops),
        # then one output DMA.
        nc.scalar.activation(
            escr[:], zbk[:], AF.Exp, bias=bc_sb[:, HT:HT + 1],
            accum_out=sexp1[:, 0:1],
        )
        nc.sync.dma_start(outS[:, :], sexp1[:, :])

        for p in (pp_d, pp_z, pp_pre, rpool, cpool):
            p.release()

    nc.finalize()
    return nc


def _get_module():
    if "nc" not in _CACHE:
        _CACHE["nc"] = _build_module()
    return _CACHE["nc"]


def kernel(**inputs) -> np.ndarray:
    import ml_dtypes
    from concourse.bass_utils import run_bass_kernel_spmd

    bf = ml_dtypes.bfloat16
    f8 = ml_dtypes.float8_e4m3
    x = np.ascontiguousarray(np.asarray(inputs["x_samples"], dtype=np.float32))
    y = np.ascontiguousarray(np.asarray(inputs["y_samples"], dtype=np.float32))
    W1 = np.asarray(inputs["W1"], dtype=np.float32)
    b1 = np.asarray(inputs["b1"], dtype=np.float32).reshape(H)
    W2 = np.asarray(inputs["W2"], dtype=np.float32)
    b2 = float(np.asarray(inputs["b2"], dtype=np.float32).reshape(1)[0])

    def sbuf_fmt(aT):
        # [KD*128, F] transposed matrix -> SBUF tile layout [128, KD*F]
        # (row p, col k*F+f = aT[128k+p, f]), contiguous.
        kd = aT.shape[0] // 128
        return np.ascontiguousarray(
            aT.reshape(kd, 128, -1).transpose(1, 0, 2).reshape(128, -1).astype(bf)
        )

    xt_p = sbuf_fmt(x.T)             # [128, 6*512]
    w1x_p = sbuf_fmt(W1[:, :XD].T)   # [128, 6*300]
    w1y_p = sbuf_fmt(W1[:, XD:].T)   # [128, 6*300]

    bcons = np.zeros((128, HT + 1), np.float32)
    w2 = W2.reshape(H)
    hs2 = HSZ[2]
    for t in range(2):
        bcons[:, t] = b1[128 * t:128 * (t + 1)]
    bcons[:hs2, 2] = b1[256:256 + hs2]
    bcons[:, 3] = b2

    # bf16 weight block: diag cols + M=64 weight-window strips (chunk at
    # col 63 of each 127/128-wide strip).
    w2a = np.zeros((128, W2A_W), np.float32)
    w2a[:, 0] = w2[0:128]
    w2a[:, 1] = w2[128:256]
    w2a[:hs2, 2] = w2[256:256 + hs2]
    w2a[:, W_T0 + 63] = w2[0:128]
    w2a[:, W_T1 + 63] = w2[128:256]
    w2a[:hs2, W_T2 + 63] = w2[256:256 + hs2]
    w2a[64:64 + hs2, W_T2 + 64] = w2[256:256 + hs2]

    # fp8 DoubleRow blocks [128, 2, 64] (slot stride 64, 16B-aligned):
    # j = 2b + a: slot0 col q=8b+ACT_I[a] <- w2[0:128], slot1 same col <-
    # w2[128:256]. Then 8 t2 blocks j2 = b_even + p: slot0 = q-pair of
    # block b_even (tail rows 0:44 / 64:108), slot1 = q-pair+8 of b_even+1.
    w2f8 = np.zeros((128, W2F8_W), np.float32)
    for b in range(8):
        for a, i_rel in enumerate(ACT_I):
            j = 2 * b + a
            q = 8 * b + i_rel
            w2f8[:, 128 * j + q] = w2[0:128]
            w2f8[:, 128 * j + 64 + q] = w2[128:256]
    for b_even in range(0, 8, 2):
        for p in (0, 1):
            j2 = b_even + p
            q = 8 * b_even + 2 * p
            w2f8[:hs2, W2F8_T2 + 128 * j2 + q] = w2[256:256 + hs2]
            w2f8[64:64 + hs2, W2F8_T2 + 128 * j2 + q + 1] = w2[256:256 + hs2]
            w2f8[:hs2, W2F8_T2 + 128 * j2 + 64 + q + 8] = w2[256:256 + hs2]
            w2f8[64:64 + hs2, W2F8_T2 + 128 * j2 + 64 + q + 9] = w2[256:256 + hs2]

    b1row = np.zeros((1, 3 * 128), np.float32)
    for t in range(2):
        b1row[0, 128 * t:128 * (t + 1)] = b1[128 * t:128 * (t + 1)]
    b1row[0, 256:256 + hs2] = b1[256:256 + hs2]

    in_maps = []
    for c in range(NCORES):
        sl = slice(c * ISH, (c + 1) * ISH)
        aux = np.concatenate(
            [sbuf_fmt(y[sl].T), sbuf_fmt(x[sl].T), w2a.astype(bf)], axis=1
        )
        in_maps.append({
            "xt_p": xt_p,
            "w1x_p": w1x_p,
            "w1y_p": w1y_p,
            "aux_p": np.ascontiguousarray(aux),
            "bcons": bcons,
            "w2f8": np.ascontiguousarray(w2f8.astype(f8)),
            "b1row": np.ascontiguousarray(b1row.astype(bf)),
        })

    nc = _get_module()
    res = run_bass_kernel_spmd(
        nc, in_maps, core_ids=list(range(NCORES)), trace=TRACE
    )
    global LAST_RESULTS
    LAST_RESULTS = res

    # device outS = accumulator column; S for local i at row i.
    S = np.concatenate(
        [r["outS"].reshape(128)[:ISH] for r in res.results]
    ).astype(np.float64)
    d = np.concatenate([r["outD"].reshape(ISH) for r in res.results]).astype(np.float64)
    v = d + b2
    t0 = np.log1p(np.exp(-np.abs(v))) + np.maximum(v, 0.0)   # softplus(diag + b2)
    lse = np.log(float(N) + S)
    val = t0.mean() - (lse.mean() - math.log(N))
    return np.float32(val)


# revision 33
# speedup vs baseline: 1.0333x; 1.0333x over previous
"""InfoNCE lower-bound kernel for 8 Trainium2 NeuronCores (v3).

Math (reference):
  hx = x @ W1x.T ; hy = y @ W1y.T            [N, H]
  z_ij = relu(hx[j] + hy[i] + b1) . w2       (logit WITHOUT b2)
  T1[i,j] = softplus(z_ij + b2)
  T0[i]   = T1[i,i]
  lse[i]  = log(sum_j exp(T1[i,j])) = log(N + sum_j exp(z_ij + b2))
  out     = mean(T0) - (mean(lse) - log N)

Sharding: data-parallel over i (rows of the pair grid); each core gets 64
rows of y, x and params replicated.

v3 design (on top of the v2 fp32->bf16 baseline):
  * All inputs pre-formatted on the host into exact SBUF layouts so every
    input DMA is a flat [128, F] contiguous copy; issues are spread across
    both HWDGE engines (sync + scalar). Each DMA issue costs ~600ns of
    sequencer time regardless of size, so transfers are consolidated.
  * PE p-state warmup: a run of dummy [128,64] matmuls burns the slow
    p-state window while input DMAs are in flight.
  * z accumulation uses TWO psum banks (M=32 weight windows): row i ->
    bank i//32, psum row q=i%32. Every z matmul writes its bank's full
    [32,512] region (w2 chunk at col 31-q..+32 of a zero strip -> local
    col q). All accumulation into a bank is single-strip sequential (the
    cross-strip concurrent case corrupts on HW). Bank 0 completes after
    block 3, so its exp runs DURING blocks 4-7; only bank 1's exp is in
    the tail.
  * exp uses the Act accumulator (accum_out) to fuse the free-axis
    sum_j exp(z+b2) into the activation op - no DVE reduces at all.
  * The relu grid is split DVE 14 / Act 6 ops per 8-row block. Act rows
    ({2,5} rel.) write fp8 PAIR tiles [128,1024] = [t0|t1] consumed by ONE
    DoubleRow fp8 matmul each (2 k-tiles per stream at 0.5 cyc/row); Act
    also covers the t2 tail pairs 0,1 as fp8 pair-of-blocks tiles (one
    DoubleRow per 2 blocks). DVE tiles stay bf16 (4x DVE mode) with plain
    bf16 matmuls. fp8 noise is ~1e-4 abs on the result vs the 7.8e-4 abs
    error budget.
  * Device ships per-row partial results (sum_j exp(z+b2) and diag
    logits); the final ln/softplus/means run on the host.
"""

import math

import numpy as np

N = 512
XD = 768
YD = 768
H = 300
NCORES = 8
ISH = N // NCORES   # 64 rows per core
KD = XD // 128      # 6 contraction tiles of 128
HT = 3              # h tiles: 128, 128, 44
HSZ = [128, 128, H - 256]

ACT_I = (2, 5)                # rel. rows per block on Act -> fp8 DoubleRow
IORD_DVE = (0, 1, 3, 4, 6, 7)

# w2a_sb (bf16) column offsets: diag cols 0:4, then M=32 weight-window
# strips (chunk at col 31; lhsT window [31-q : 63-q] -> local col q).
W_T0, W_T1, W_T2 = 4, 67, 130
W2A_W = 194
# w2f8_sb (fp8 DoubleRow): windows are illegal (LdWeights dual-fp8 needs
# 16B-aligned APs with slot stride %16==0), so per-q blocks [128, 2, 32]:
# 8 t0|t1 blocks (key 2*(b//2)+a, shared by both block parities) then 4 t2
# pair-of-blocks blocks (key 2*(b//4)+p).
W2F8_T2 = 8 * 64
W2F8_W = 12 * 64

_CACHE = {}
TRACE = False
LAST_RESULTS = None


def _build_module():
    import concourse.bacc as bacc
    import concourse.mybir as mybir
    from concourse.tile import TileContext

    f32 = mybir.dt.float32
    bf16 = mybir.dt.bfloat16
    f8 = mybir.dt.float8e4
    AF = mybir.ActivationFunctionType
    ALU = mybir.AluOpType
    DROW = mybir.MatmulPerfMode.DoubleRow

    nc = bacc.Bacc("TRN2", target_bir_lowering=False, debug=False)

    # Per-core inputs (SPMD: same shapes, different data for yt/xtd slices).
    xt_p = nc.dram_tensor("xt_p", [128, KD * N], bf16, kind="ExternalInput")
    w1x_p = nc.dram_tensor("w1x_p", [128, KD * H], bf16, kind="ExternalInput")
    w1y_p = nc.dram_tensor("w1y_p", [128, KD * H], bf16, kind="ExternalInput")
    aux_p = nc.dram_tensor("aux_p", [128, 2 * KD * ISH + W2A_W], bf16, kind="ExternalInput")
    bcons = nc.dram_tensor("bcons", [128, HT + 1], f32, kind="ExternalInput")
    w2f8 = nc.dram_tensor("w2f8", [128, W2F8_W], f8, kind="ExternalInput")
    b1row = nc.dram_tensor("b1row", [1, 3 * 128], bf16, kind="ExternalInput")
    outS = nc.dram_tensor("outS", [128, 2], f32, kind="ExternalOutput")  # accum image
    outD = nc.dram_tensor("outD", [1, ISH], f32, kind="ExternalOutput")  # diag logits

    AUX_YT, AUX_XTD, AUX_W2A = 0, KD * ISH, 2 * KD * ISH

    with TileContext(nc) as tc:
        cpool = tc.alloc_tile_pool(name="consts", bufs=1)
        rpool = tc.alloc_tile_pool(name="work", bufs=32)
        pp_pre = tc.alloc_tile_pool(name="pp_pre", bufs=1, space="PSUM")
        pp_z = tc.alloc_tile_pool(name="pp_z", bufs=1, space="PSUM")
        pp_d = tc.alloc_tile_pool(name="pp_d", bufs=1, space="PSUM")

        # ---- constant tiles ----
        xt_sb = cpool.tile([128, KD * N], bf16, tag="xt")
        w1x_sb = cpool.tile([128, KD * H], bf16, tag="w1x")
        w1y_sb = cpool.tile([128, KD * H], bf16, tag="w1y")
        aux_sb = cpool.tile([128, 2 * KD * ISH + W2A_W], bf16, tag="aux")
        bc_sb = cpool.tile([128, HT + 1], f32, tag="bc")
        w2f8_sb = cpool.tile([128, W2F8_W], f8, tag="w2f8")
        b1r_sb = cpool.tile([1, 3 * 128], bf16, tag="b1r")

        hxb0 = cpool.tile([128, N], bf16, tag="hxb0")    # relu-arg x part (+b1), t0
        hxb1 = cpool.tile([128, N], bf16, tag="hxb1")    # t1
        hxb2p = cpool.tile([128, N], bf16, tag="hxb2p")  # t2 packed (rows 0:44, 64:108)
        hyf_sb = cpool.tile([128, HT * ISH], f32, tag="hyf")   # hy per h-tile (f32)
        hy2p = cpool.tile([128, ISH // 2], f32, tag="hy2p")    # packed t2 pairs
        ones64 = cpool.tile([1, ISH], bf16, tag="ones64")
        escr = cpool.tile([128, 2 * N], bf16, tag="escr")      # exp out (unread)
        sexp2 = cpool.tile([128, 2], f32, tag="sexp2")
        dlog = cpool.tile([1, ISH], f32, tag="dlog")
        warm = cpool.tile([128, 64], bf16, tag="warm")

        def w2a(lo, hi):
            return aux_sb[:, AUX_W2A + lo:AUX_W2A + hi]

        # ---- input DMAs ----
        def half(eng, dst, src, h):
            w = dst.shape[1] // 2
            eng.dma_start(dst[:, h * w:(h + 1) * w], src[:, h * w:(h + 1) * w])
        half(nc.sync, xt_sb, xt_p, 0)
        half(nc.scalar, w1x_sb, w1x_p, 0)
        half(nc.sync, xt_sb, xt_p, 1)
        half(nc.scalar, w1x_sb, w1x_p, 1)
        nc.sync.dma_start(aux_sb[:], aux_p[:])
        nc.scalar.dma_start(w1y_sb[:], w1y_p[:])
        nc.sync.dma_start(bc_sb[:], bcons[:])
        nc.scalar.dma_start(w2f8_sb[:], w2f8[:])
        nc.sync.dma_start(b1r_sb[:], b1row[:])

        # ---- PE p-state warmup: burn the slow-clock window on dummy
        # matmuls (no input deps) while the DMAs land ----
        nc.gpsimd.memset(warm[:], 0.0)
        wps = pp_d.tile([128, 64], f32, tag="warm_ps")
        for _ in range(56):
            nc.tensor.matmul(wps[0:64, :], lhsT=warm[:, 0:64], rhs=warm[:],
                             start=True, stop=True, skip_group_check=True)

        nc.gpsimd.memset(ones64[:], 1.0)
        # zero the packed-t2 operand tiles before their writers fill the
        # live rows, so the pair matmul's zero-weight rows multiply finite
        # values (NaN * 0 = NaN).
        nc.gpsimd.memset(hxb2p[:], 0.0)
        nc.gpsimd.memset(hy2p[:], 0.0)

        # ---- preamble: hxb (x part, +b1) and hy interleaved per h-tile.
        # PSUM->SBUF copies go through Act (idle during the preamble) so the
        # DVE spends its time on the relu grid only. ----
        for t in range(HT):
            hs = HSZ[t]
            ps = pp_pre.tile([128, N], f32, tag="pre512", bufs=2)
            for k in range(KD):
                nc.tensor.matmul(
                    ps[0:hs, :],
                    lhsT=w1x_sb[:, k * H + 128 * t: k * H + 128 * t + hs],
                    rhs=xt_sb[:, k * N:(k + 1) * N],
                    start=(k == 0), stop=(k == KD - 1),
                )
            dst = [hxb0, hxb1, hxb2p][t]
            nc.scalar.activation(
                dst[0:hs, :], ps[0:hs, :], AF.Identity, bias=bc_sb[0:hs, t:t + 1]
            )
            psy = pp_pre.tile([128, ISH], f32, tag="pre64", bufs=1)
            for k in range(KD):
                nc.tensor.matmul(
                    psy[0:hs, :],
                    lhsT=w1y_sb[:, k * H + 128 * t: k * H + 128 * t + hs],
                    rhs=aux_sb[:, AUX_YT + k * ISH:AUX_YT + (k + 1) * ISH],
                    start=(k == 0), stop=(k == KD - 1),
                )
            nc.scalar.activation(
                hyf_sb[0:hs, t * ISH:(t + 1) * ISH], psy[0:hs, :], AF.Identity
            )
            if t == 2:
                # packed pair layout: col p <- (even col 2p at rows 0:44,
                # odd col 2p+1 at rows 64:108)
                evens = psy[0:hs, :].rearrange("p (a two) -> p two a", two=2)
                nc.scalar.activation(hy2p[0:hs, :], evens[:, 0, :], AF.Identity)
                nc.scalar.activation(hy2p[64:64 + hs, :], evens[:, 1, :], AF.Identity)
        nc.vector.tensor_copy(hxb2p[64:64 + HSZ[2], :], hxb2p[0:HSZ[2], :])

        # ---- main loop: 8 blocks of 8 rows over TWO psum banks, both at
        # PE position (0,0) (DoubleRow forbids column tiling) ----
        # Row i (local) -> bank (i//8)%2, psum row q = 8*(i//16) + i%8.
        # Blocks are processed in pairs (2k, 2k+1) with their z matmuls
        # interleaved A,B,A,B..., so consecutive matmuls never accumulate
        # into the same psum region back-to-back (same-region back-to-back
        # costs ~70ns/matmul of accumulate-drain hazard, measured).
        def relu_op(eng, out_ap, in_ap, col_f32):
            if eng == 'A':
                nc.scalar.activation(out_ap, in_ap, AF.Relu, bias=col_f32)
            else:
                nc.vector.tensor_scalar(out_ap, in_ap, col_f32, 0.0, ALU.add, ALU.max)

        zbk = [
            pp_z.tile([128, N], f32, tag=f"zp{c}", name=f"zp{c}") for c in range(2)
        ]

        NB = ISH // 8
        rp2t = {}   # (parity, p) -> fp8 t2 tile paired across (b, b+2)
        rt = {}     # (b, i_rel, t) -> bf16 tile (DVE rows)
        rf8 = {}    # (b, i_rel) -> fp8 [128, 1024] pair tile (Act rows)
        rps = {}    # (b, p) -> bf16 t2 pair tile (DVE pairs 2, 3)
        for k in range(NB // 2):
            pair = (2 * k, 2 * k + 1)
            # --- relu ops, interleaved across the block pair ---
            for t in range(2):
                for i_rel in IORD_DVE:
                    for b in pair:
                        r = rpool.tile([128, N], bf16, tag="r", bufs=30,
                                       name=f"r_{b}_{i_rel}_{t}")
                        relu_op(
                            'D', r[:], [hxb0, hxb1][t][:],
                            hyf_sb[:, t * ISH + 8 * b + i_rel:
                                   t * ISH + 8 * b + i_rel + 1],
                        )
                        rt[(b, i_rel, t)] = r
            for i_rel in ACT_I:
                for b in pair:
                    rf = rpool.tile([128, 2 * N], f8, tag="rf8", bufs=6,
                                    name=f"rf8_{b}_{i_rel}")
                    for t in range(2):
                        relu_op(
                            'A', rf[:, t * N:(t + 1) * N], [hxb0, hxb1][t][:],
                            hyf_sb[:, t * ISH + 8 * b + i_rel:
                                   t * ISH + 8 * b + i_rel + 1],
                        )
                    rf8[(b, i_rel)] = rf
            for p in (0, 1):           # Act fp8 t2, paired across (b, b+2)
                for b in pair:
                    if k % 2 == 0:
                        rp2t[(b % 2, p)] = rpool.tile(
                            [128, 2 * N], f8, tag="rp2", bufs=6,
                            name=f"rp2_{b}_{p}")
                    relu_op('A', rp2t[(b % 2, p)][:, (k % 2) * N:(k % 2 + 1) * N],
                            hxb2p[:], hy2p[:, 4 * b + p:4 * b + p + 1])
            for p in (2, 3):           # DVE bf16 t2 pairs
                for b in pair:
                    rp = rpool.tile([128, N], bf16, tag="rp", bufs=6,
                                    name=f"rp_{b}_{p}")
                    relu_op('D', rp[:], hxb2p[:], hy2p[:, 4 * b + p:4 * b + p + 1])
                    rps[(b, p)] = rp

            # --- z matmuls, banks interleaved A,B,A,B... ---
            def zmm(b, i_rel, t, start=False):
                q = 8 * (b // 2) + i_rel
                base = [W_T0, W_T1][t]
                nc.tensor.matmul(
                    zbk[b % 2][0:32, :], lhsT=w2a(base + 31 - q, base + 63 - q),
                    rhs=rt[(b, i_rel, t)][:], start=start, stop=False,
                    skip_group_check=True,
                )

            def zdr(b, i_rel):
                j = 2 * (b // 2) + (0 if i_rel == ACT_I[0] else 1)
                nc.tensor.matmul(
                    zbk[b % 2][0:32, :],
                    lhsT=w2f8_sb[:, 64 * j:64 * (j + 1)]
                    .rearrange("p (two m) -> p two m", two=2),
                    rhs=rf8[(b, i_rel)][:].rearrange("p (two f) -> p two f", two=2),
                    start=False, stop=False, perf_mode=DROW,
                    skip_group_check=True,
                )

            def zpair(b, p):
                q = 8 * (b // 2) + 2 * p
                nc.tensor.matmul(
                    zbk[b % 2][0:32, :], lhsT=w2a(W_T2 + 31 - q, W_T2 + 63 - q),
                    rhs=rps[(b, p)][:], start=False, stop=False,
                    skip_group_check=True,
                )

            def zdr_t2(b, p, stop):
                j2 = 2 * (b // 4) + p          # blocks (b-2, b), pair p
                nc.tensor.matmul(
                    zbk[b % 2][0:32, :],
                    lhsT=w2f8_sb[:, W2F8_T2 + 64 * j2:W2F8_T2 + 64 * (j2 + 1)]
                    .rearrange("p (two m) -> p two m", two=2),
                    rhs=rp2t[(b % 2, p)][:].rearrange("p (two f) -> p two f", two=2),
                    start=False, stop=stop, perf_mode=DROW,
                    skip_group_check=True,
                )

            for t in range(2):
                for i_rel in IORD_DVE:
                    for b in pair:
                        zmm(b, i_rel, t,
                            start=(k == 0 and t == 0 and i_rel == IORD_DVE[0]))
            for i_rel in ACT_I:
                for b in pair:
                    zdr(b, i_rel)
            for p in (2, 3):
                for b in pair:
                    zpair(b, p)
            if k % 2 == 1:
                for p in (0, 1):
                    for b in pair:
                        zdr_t2(b, p, stop=(k == 3 and p == 1))

            if k == 3:
                # both banks complete: exp(z + b2) with fused free-axis
                # accumulate sum_j. Valid rows 0:32; the rest is junk the
                # host drops.
                for c in range(2):
                    nc.scalar.activation(
                        escr[:, c * N:(c + 1) * N], zbk[c][:], AF.Exp,
                        bias=bc_sb[:, HT:HT + 1],
                        accum_out=sexp2[:, c:c + 1],
                    )

            if pair[0] == 2:
                # ---- diag: dlog[i] = w2 . relu(hxd_i + hy_i + b1), fed
                # straight from PSUM (b1 added via a rank-1 K=1 matmul);
                # at b==2 so the aux/b1row DMAs land after grid start ----
                dps = pp_d.tile([128, ISH], f32, tag="dps")
                for dt_ in range(HT):
                    hs = HSZ[dt_]
                    psd = pp_pre.tile([128, ISH], f32, tag="pre64", bufs=1)
                    for k in range(KD):
                        nc.tensor.matmul(
                            psd[0:hs, :],
                            lhsT=w1x_sb[:, k * H + 128 * dt_: k * H + 128 * dt_ + hs],
                            rhs=aux_sb[:, AUX_XTD + k * ISH:AUX_XTD + (k + 1) * ISH],
                            start=(k == 0), stop=False,
                        )
                    nc.tensor.matmul(
                        psd[0:hs, :],
                        lhsT=b1r_sb[0:1, 128 * dt_:128 * dt_ + hs],
                        rhs=ones64[0:1, :],
                        start=False, stop=True,
                    )
                    dsum = rpool.tile([128, ISH], bf16, tag="dsum", bufs=2)
                    nc.vector.tensor_tensor(
                        dsum[0:hs, :], psd[0:hs, :],
                        hyf_sb[0:hs, dt_ * ISH:(dt_ + 1) * ISH], op=ALU.add,
                    )
                    dr = rpool.tile([128, ISH], bf16, tag="dr", bufs=2)
                    nc.vector.tensor_scalar(
                        dr[0:hs, :], dsum[0:hs, :], 0.0, None, ALU.max
                    )
                    dcol = AUX_W2A + (dt_ if dt_ < 2 else 2)
                    nc.tensor.matmul(
                        dps[0:1, :],
                        lhsT=aux_sb[0:hs, dcol:dcol + 1],
                        rhs=dr[0:hs, :],
                        start=(dt_ == 0), stop=(dt_ == HT - 1),
                    )
                nc.scalar.activation(dlog[0:1, :], dps[0:1, :], AF.Identity)
                nc.sync.dma_start(outD[0:1, :], dlog[0:1, :])

        # one output DMA for both banks' accumulators.
        nc.sync.dma_start(outS[:, :], sexp2[:, :])

        for p in (pp_d, pp_z, pp_pre, rpool, cpool):
            p.release()

    nc.finalize()
    return nc


def _get_module():
    if "nc" not in _CACHE:
        _CACHE["nc"] = _build_module()
    return _CACHE["nc"]


def kernel(**inputs) -> np.ndarray:
    import ml_dtypes
    from concourse.bass_utils import run_bass_kernel_spmd

    bf = ml_dtypes.bfloat16
    f8 = ml_dtypes.float8_e4m3
    x = np.ascontiguousarray(np.asarray(inputs["x_samples"], dtype=np.float32))
    y = np.ascontiguousarray(np.asarray(inputs["y_samples"], dtype=np.float32))
    W1 = np.asarray(inputs["W1"], dtype=np.float32)
    b1 = np.asarray(inputs["b1"], dtype=np.float32).reshape(H)
    W2 = np.asarray(inputs["W2"], dtype=np.float32)
    b2 = float(np.asarray(inputs["b2"], dtype=np.float32).reshape(1)[0])

    def sbuf_fmt(aT):
        # [KD*128, F] transposed matrix -> SBUF tile layout [128, KD*F]
        # (row p, col k*F+f = aT[128k+p, f]), contiguous.
        kd = aT.shape[0] // 128
        return np.ascontiguousarray(
            aT.reshape(kd, 128, -1).transpose(1, 0, 2).reshape(128, -1).astype(bf)
        )

    xt_p = sbuf_fmt(x.T)             # [128, 6*512]
    w1x_p = sbuf_fmt(W1[:, :XD].T)   # [128, 6*300]
    w1y_p = sbuf_fmt(W1[:, XD:].T)   # [128, 6*300]

    bcons = np.zeros((128, HT + 1), np.float32)
    w2 = W2.reshape(H)
    hs2 = HSZ[2]
    for t in range(2):
        bcons[:, t] = b1[128 * t:128 * (t + 1)]
    bcons[:hs2, 2] = b1[256:256 + hs2]
    bcons[:, 3] = b2

    # bf16 weight block: diag cols + M=32 weight-window strips (chunk at
    # col 31 of each 63/64-wide strip).
    w2a = np.zeros((128, W2A_W), np.float32)
    w2a[:, 0] = w2[0:128]
    w2a[:, 1] = w2[128:256]
    w2a[:hs2, 2] = w2[256:256 + hs2]
    w2a[:, W_T0 + 31] = w2[0:128]
    w2a[:, W_T1 + 31] = w2[128:256]
    w2a[:hs2, W_T2 + 31] = w2[256:256 + hs2]
    w2a[64:64 + hs2, W_T2 + 32] = w2[256:256 + hs2]

    # fp8 DoubleRow blocks [128, 2, 32] (slot stride 32, 16B-aligned):
    # t0|t1 block j = 2*kq + a: slot0 col q=8*kq+ACT_I[a] <- w2[0:128],
    # slot1 same col <- w2[128:256] (shared by both block parities). Then
    # 4 t2 blocks j2 = 2*kk + p: slot0 = q-pair (16*kk + 2p) tail rows
    # 0:44 / 64:108, slot1 = q-pair+8 (the (b, b+2) partner block).
    w2f8 = np.zeros((128, W2F8_W), np.float32)
    for kq in range(4):
        for a, i_rel in enumerate(ACT_I):
            j = 2 * kq + a
            q = 8 * kq + i_rel
            w2f8[:, 64 * j + q] = w2[0:128]
            w2f8[:, 64 * j + 32 + q] = w2[128:256]
    for kk in range(2):
        for p in (0, 1):
            j2 = 2 * kk + p
            q = 16 * kk + 2 * p
            w2f8[:hs2, W2F8_T2 + 64 * j2 + q] = w2[256:256 + hs2]
            w2f8[64:64 + hs2, W2F8_T2 + 64 * j2 + q + 1] = w2[256:256 + hs2]
            w2f8[:hs2, W2F8_T2 + 64 * j2 + 32 + q + 8] = w2[256:256 + hs2]
            w2f8[64:64 + hs2, W2F8_T2 + 64 * j2 + 32 + q + 9] = w2[256:256 + hs2]

    b1row = np.zeros((1, 3 * 128), np.float32)
    for t in range(2):
        b1row[0, 128 * t:128 * (t + 1)] = b1[128 * t:128 * (t + 1)]
    b1row[0, 256:256 + hs2] = b1[256:256 + hs2]

    in_maps = []
    for c in range(NCORES):
        sl = slice(c * ISH, (c + 1) * ISH)
        aux = np.concatenate(
            [sbuf_fmt(y[sl].T), sbuf_fmt(x[sl].T), w2a.astype(bf)], axis=1
        )
        in_maps.append({
            "xt_p": xt_p,
            "w1x_p": w1x_p,
            "w1y_p": w1y_p,
            "aux_p": np.ascontiguousarray(aux),
            "bcons": bcons,
            "w2f8": np.ascontiguousarray(w2f8.astype(f8)),
            "b1row": np.ascontiguousarray(b1row.astype(bf)),
        })

    nc = _get_module()
    res = run_bass_kernel_spmd(
        nc, in_maps, core_ids=list(range(NCORES)), trace=TRACE
    )
    global LAST_RESULTS
    LAST_RESULTS = res

    # device outS[:, bank]: S for local i at row 8*(i//16) + i%8 of
    # col (i//8)%2.
    ii = np.arange(ISH)
    S = np.concatenate(
        [r["outS"].reshape(128, 2)[8 * (ii // 16) + ii % 8, (ii // 8) % 2]
         for r in res.results]
    ).astype(np.float64)
    d = np.concatenate([r["outD"].reshape(ISH) for r in res.results]).astype(np.float64)
    v = d + b2
    t0 = np.log1p(np.exp(-np.abs(v))) + np.maximum(v, 0.0)   # softplus(diag + b2)
    lse = np.log(float(N) + S)
    val = t0.mean() - (lse.mean() - math.log(N))
    return np.float32(val)


# revision 45
# speedup vs baseline: 1.2367x; 1.1967x over previous
"""InfoNCE lower-bound kernel for 8 Trainium2 NeuronCores (v3).

Math (reference):
  hx = x @ W1x.T ; hy = y @ W1y.T            [N, H]
  z_ij = relu(hx[j] + hy[i] + b1) . w2       (logit WITHOUT b2)
  T1[i,j] = softplus(z_ij + b2)
  T0[i]   = T1[i,i]
  lse[i]  = log(sum_j exp(T1[i,j])) = log(N + sum_j exp(z_ij + b2))
  out     = mean(T0) - (mean(lse) - log N)

Sharding: data-parallel over i (rows of the pair grid); each core gets 64
rows of y, x and params replicated.

v3 design (on top of the v2 fp32->bf16 baseline):
  * All inputs pre-formatted on the host into exact SBUF layouts so every
    input DMA is a flat [128, F] contiguous copy; issues are spread across
    both HWDGE engines (sync + scalar). Each DMA issue costs ~600ns of
    sequencer time regardless of size, so transfers are consolidated.
  * PE p-state warmup: a run of dummy [128,64] matmuls burns the slow
    p-state window while input DMAs are in flight.
  * z accumulation uses TWO psum banks (M=32 weight windows): row i ->
    bank i//32, psum row q=i%32. Every z matmul writes its bank's full
    [32,512] region (w2 chunk at col 31-q..+32 of a zero strip -> local
    col q). All accumulation into a bank is single-strip sequential (the
    cross-strip concurrent case corrupts on HW). Bank 0 completes after
    block 3, so its exp runs DURING blocks 4-7; only bank 1's exp is in
    the tail.
  * exp uses the Act accumulator (accum_out) to fuse the free-axis
    sum_j exp(z+b2) into the activation op - no DVE reduces at all.
  * The relu grid is split DVE 14 / Act 6 ops per 8-row block. Act rows
    ({2,5} rel.) write fp8 PAIR tiles [128,1024] = [t0|t1] consumed by ONE
    DoubleRow fp8 matmul each (2 k-tiles per stream at 0.5 cyc/row); Act
    also covers the t2 tail pairs 0,1 as fp8 pair-of-blocks tiles (one
    DoubleRow per 2 blocks). DVE tiles stay bf16 (4x DVE mode) with plain
    bf16 matmuls. fp8 noise is ~1e-4 abs on the result vs the 7.8e-4 abs
    error budget.
  * Device ships per-row partial results (sum_j exp(z+b2) and diag
    logits); the final ln/softplus/means run on the host.
"""

import math

import numpy as np

N = 512
XD = 768
YD = 768
H = 300
NCORES = 8
ISH = N // NCORES   # 64 rows per core
KD = XD // 128      # 6 contraction tiles of 128
HT = 3              # h tiles: 128, 128, 44
HSZ = [128, 128, H - 256]

# Engine assignment for the 20 relu ops per block of 8 rows, in emission
# order [8x t0, 8x t1, 4x t2-pair]. 'D' = DVE (vector), 'A' = Activation
# (scalar). fp8 DoubleRow was tried and REVERTED: DoubleRow forbids the
# tile_position column tiling below, and that tiling is worth more — PE
# matmuls on different column quadrants genuinely overlap (measured
# ~171ns effective per z matmul vs 216 without, and DR itself measured
# 445ns vs its 2x216 replacement).
PATTERN20_6A = (['D', 'A', 'D', 'D', 'D', 'D', 'A', 'D'] * 2 + ['A', 'A', 'D', 'D'])
PATTERN20_5A = (['D', 'A', 'D', 'D', 'D', 'D', 'A', 'D'] * 2 + ['A', 'D', 'D', 'D'])

# w2a_sb (bf16) column offsets: diag cols 0:4, then 40 16-col q-blocks:
# sec0 = t0 (q=0..15), sec1 = t1, sec2 = t2 pairs (m=0..7: w2 tail in col
# 2m rows 0:44 and col 2m+1 rows 64:108).
W2A_W = 4 + 40 * 16

_CACHE = {}
TRACE = False
LAST_RESULTS = None


def _build_module():
    import concourse.bacc as bacc
    import concourse.mybir as mybir
    from concourse.tile import TileContext

    f32 = mybir.dt.float32
    bf16 = mybir.dt.bfloat16
    AF = mybir.ActivationFunctionType
    ALU = mybir.AluOpType

    nc = bacc.Bacc("TRN2", target_bir_lowering=False, debug=False)

    # Per-core inputs (SPMD: same shapes, different data for yt/xtd slices).
    xt_p = nc.dram_tensor("xt_p", [128, KD * N], bf16, kind="ExternalInput")
    w1x_p = nc.dram_tensor("w1x_p", [128, KD * H], bf16, kind="ExternalInput")
    w1y_p = nc.dram_tensor("w1y_p", [128, KD * H], bf16, kind="ExternalInput")
    aux_p = nc.dram_tensor("aux_p", [128, 2 * KD * ISH + W2A_W], bf16, kind="ExternalInput")
    bcons = nc.dram_tensor("bcons", [128, HT + 1], f32, kind="ExternalInput")
    b1row = nc.dram_tensor("b1row", [1, 3 * 128], bf16, kind="ExternalInput")
    outS = nc.dram_tensor("outS", [128, 4], f32, kind="ExternalOutput")  # accum image
    outD = nc.dram_tensor("outD", [1, ISH], f32, kind="ExternalOutput")  # diag logits

    AUX_YT, AUX_XTD, AUX_W2A = 0, KD * ISH, 2 * KD * ISH

    with TileContext(nc) as tc:
        cpool = tc.alloc_tile_pool(name="consts", bufs=1)
        rpool = tc.alloc_tile_pool(name="work", bufs=32)
        pp_pre = tc.alloc_tile_pool(name="pp_pre", bufs=1, space="PSUM")
        pp_z = tc.alloc_tile_pool(name="pp_z", bufs=1, space="PSUM")
        pp_d = tc.alloc_tile_pool(name="pp_d", bufs=1, space="PSUM")

        # ---- constant tiles ----
        xt_sb = cpool.tile([128, KD * N], bf16, tag="xt")
        w1x_sb = cpool.tile([128, KD * H], bf16, tag="w1x")
        w1y_sb = cpool.tile([128, KD * H], bf16, tag="w1y")
        aux_sb = cpool.tile([128, 2 * KD * ISH + W2A_W], bf16, tag="aux")
        bc_sb = cpool.tile([128, HT + 1], f32, tag="bc")
        b1r_sb = cpool.tile([1, 3 * 128], bf16, tag="b1r")

        hxb0 = cpool.tile([128, N], bf16, tag="hxb0")    # relu-arg x part (+b1), t0
        hxb1 = cpool.tile([128, N], bf16, tag="hxb1")    # t1
        hxb2p = cpool.tile([128, N], bf16, tag="hxb2p")  # t2 packed (rows 0:44, 64:108)
        hyf_sb = cpool.tile([128, HT * ISH], f32, tag="hyf")   # hy per h-tile (f32)
        hy2p = cpool.tile([128, ISH // 2], f32, tag="hy2p")    # packed t2 pairs
        ones64 = cpool.tile([1, ISH], bf16, tag="ones64")
        escr = cpool.tile([128, 4 * N], bf16, tag="escr")      # exp out (unread)
        sexp4 = cpool.tile([128, 4], f32, tag="sexp4")
        dlog = cpool.tile([1, ISH], f32, tag="dlog")
        warm = cpool.tile([128, 64], bf16, tag="warm")

        def w2a(lo, hi):
            return aux_sb[:, AUX_W2A + lo:AUX_W2A + hi]

        # ---- input DMAs ----
        def half(eng, dst, src, h):
            w = dst.shape[1] // 2
            eng.dma_start(dst[:, h * w:(h + 1) * w], src[:, h * w:(h + 1) * w])
        half(nc.sync, xt_sb, xt_p, 0)
        half(nc.scalar, w1x_sb, w1x_p, 0)
        half(nc.sync, xt_sb, xt_p, 1)
        half(nc.scalar, w1x_sb, w1x_p, 1)
        nc.sync.dma_start(aux_sb[:], aux_p[:])
        nc.scalar.dma_start(w1y_sb[:], w1y_p[:])
        nc.sync.dma_start(bc_sb[:], bcons[:])
        nc.scalar.dma_start(b1r_sb[:], b1row[:])

        # ---- PE p-state warmup: burn the slow-clock window on dummy
        # matmuls (no input deps) while the DMAs land ----
        nc.gpsimd.memset(warm[:], 0.0)
        # shares the diag dps buffer (same pool/tag/shape) — psum is full
        wps = pp_d.tile([128, 64], f32, tag="dps", name="warm_ps")
        for _ in range(56):
            nc.tensor.matmul(wps[0:64, :], lhsT=warm[:, 0:64], rhs=warm[:],
                             start=True, stop=True, skip_group_check=True)

        nc.gpsimd.memset(ones64[:], 1.0)
        # zero the packed-t2 operand tiles before their writers fill the
        # live rows, so the pair matmul's zero-weight rows multiply finite
        # values (NaN * 0 = NaN).
        nc.gpsimd.memset(hxb2p[:], 0.0)
        nc.gpsimd.memset(hy2p[:], 0.0)

        # ---- preamble: hxb (x part, +b1) and hy interleaved per h-tile.
        # PSUM->SBUF copies go through Act (idle during the preamble) so the
        # DVE spends its time on the relu grid only. ----
        for t in range(HT):
            hs = HSZ[t]
            ps = pp_pre.tile([128, N], f32, tag="pre512", bufs=2)
            for k in range(KD):
                nc.tensor.matmul(
                    ps[0:hs, :],
                    lhsT=w1x_sb[:, k * H + 128 * t: k * H + 128 * t + hs],
                    rhs=xt_sb[:, k * N:(k + 1) * N],
                    start=(k == 0), stop=(k == KD - 1),
                )
            dst = [hxb0, hxb1, hxb2p][t]
            nc.scalar.activation(
                dst[0:hs, :], ps[0:hs, :], AF.Identity, bias=bc_sb[0:hs, t:t + 1]
            )
            psy = pp_pre.tile([128, ISH], f32, tag="pre64", bufs=1)
            for k in range(KD):
                nc.tensor.matmul(
                    psy[0:hs, :],
                    lhsT=w1y_sb[:, k * H + 128 * t: k * H + 128 * t + hs],
                    rhs=aux_sb[:, AUX_YT + k * ISH:AUX_YT + (k + 1) * ISH],
                    start=(k == 0), stop=(k == KD - 1),
                )
            nc.scalar.activation(
                hyf_sb[0:hs, t * ISH:(t + 1) * ISH], psy[0:hs, :], AF.Identity
            )
            if t == 2:
                # packed pair layout: col p <- (even col 2p at rows 0:44,
                # odd col 2p+1 at rows 64:108)
                evens = psy[0:hs, :].rearrange("p (a two) -> p two a", two=2)
                nc.scalar.activation(hy2p[0:hs, :], evens[:, 0, :], AF.Identity)
                nc.scalar.activation(hy2p[64:64 + hs, :], evens[:, 1, :], AF.Identity)
        nc.vector.tensor_copy(hxb2p[64:64 + HSZ[2], :], hxb2p[0:HSZ[2], :])

        # ---- main loop: 8 blocks of 8 rows, v2's 4-strip z engine ----
        # row i -> strip c=(i//2)%4, psum row 32c + q, q = 2*(i//8) + i%2.
        # Consecutive z matmuls target different PE column quadrants
        # (tile_position (0,32c)) and different psum banks, which lets the
        # PE overlap their streams (~171ns effective vs 216 serial).
        def relu_op(eng, out_ap, in_ap, col_f32):
            if eng == 'A':
                nc.scalar.activation(out_ap, in_ap, AF.Relu, bias=col_f32)
            else:
                nc.vector.tensor_scalar(out_ap, in_ap, col_f32, 0.0, ALU.add, ALU.max)

        def w2q_blk(sec, idx):
            off = AUX_W2A + 4 + (sec * 16 + idx) * 16
            return aux_sb[:, off:off + 16]

        zbk = [
            pp_z.tile([128, N], f32, tag=f"zp{c}", name=f"zp{c}") for c in range(4)
        ]

        NB = ISH // 8
        for b in range(NB):
            last = b == NB - 1
            PATTERN20 = PATTERN20_6A if b < 6 else PATTERN20_5A
            if not last:
                iord = [8 * b + 2 * c + j for j in range(2) for c in range(4)]
                slots = [(i, 0) for i in iord] + [(i, 1) for i in iord] \
                    + [(c, 2) for c in range(4)]
            else:
                slots = []
                for c in range(4):
                    i0 = 8 * b + 2 * c
                    slots += [(i0, 0), (i0 + 1, 0), (i0, 1), (i0 + 1, 1), (c, 2)]
            rt = {}
            rps = {}
            for slot, (idx, t) in enumerate(slots):
                if t < 2:
                    r = rpool.tile([128, N], bf16, tag="r", bufs=32)
                    relu_op(
                        PATTERN20[slot], r[:], [hxb0, hxb1][t][:],
                        hyf_sb[:, t * ISH + idx: t * ISH + idx + 1],
                    )
                    rt[(idx, t)] = r
                else:
                    rp = rpool.tile([128, N], bf16, tag="rp", bufs=10)
                    relu_op(PATTERN20[slot], rp[:], hxb2p[:],
                            hy2p[:, 4 * b + idx:4 * b + idx + 1])
                    rps[idx] = rp

            def zmm(i, t):
                c = (i // 2) % 4
                q = 2 * (i // 8) + (i % 2)
                nc.tensor.matmul(
                    zbk[c][32 * c:32 * c + 16, :],
                    lhsT=w2q_blk(t, q), rhs=rt[(i, t)][:],
                    start=(b == 0 and t == 0 and i % 2 == 0), stop=False,
                    tile_position=(0, 32 * c),
                    skip_group_check=True,
                )

            def zpair(c):
                nc.tensor.matmul(
                    zbk[c][32 * c:32 * c + 16, :],
                    lhsT=w2q_blk(2, b), rhs=rps[c][:],
                    start=False, stop=last,
                    tile_position=(0, 32 * c),
                    skip_group_check=True,
                )

            if not last:
                for t in range(2):
                    for i in iord:
                        zmm(i, t)
                for c in range(4):
                    zpair(c)
            else:
                # strip-major drain: each strip's stop fires early so its
                # exp(+fused accumulate sum_j) pipelines behind the rest.
                for c in range(4):
                    i0 = 8 * b + 2 * c
                    zmm(i0, 0)
                    zmm(i0 + 1, 0)
                    zmm(i0, 1)
                    zmm(i0 + 1, 1)
                    zpair(c)
                    nc.scalar.activation(
                        escr[:, c * N:(c + 1) * N], zbk[c][:], AF.Exp,
                        bias=bc_sb[:, HT:HT + 1],
                        accum_out=sexp4[:, c:c + 1],
                    )

            if b == 2:
                # ---- diag: dlog[i] = w2 . relu(hxd_i + hy_i + b1), fed
                # straight from PSUM (b1 added via a rank-1 K=1 matmul);
                # at b==2 so the aux/b1row DMAs land after grid start ----
                dps = pp_d.tile([128, ISH], f32, tag="dps")
                for dt_ in range(HT):
                    hs = HSZ[dt_]
                    psd = pp_pre.tile([128, ISH], f32, tag="pre64", bufs=1)
                    for k in range(KD):
                        nc.tensor.matmul(
                            psd[0:hs, :],
                            lhsT=w1x_sb[:, k * H + 128 * dt_: k * H + 128 * dt_ + hs],
                            rhs=aux_sb[:, AUX_XTD + k * ISH:AUX_XTD + (k + 1) * ISH],
                            start=(k == 0), stop=False,
                        )
                    nc.tensor.matmul(
                        psd[0:hs, :],
                        lhsT=b1r_sb[0:1, 128 * dt_:128 * dt_ + hs],
                        rhs=ones64[0:1, :],
                        start=False, stop=True,
                    )
                    dsum = rpool.tile([128, ISH], bf16, tag="dsum", bufs=2)
                    nc.vector.tensor_tensor(
                        dsum[0:hs, :], psd[0:hs, :],
                        hyf_sb[0:hs, dt_ * ISH:(dt_ + 1) * ISH], op=ALU.add,
                    )
                    dr = rpool.tile([128, ISH], bf16, tag="dr", bufs=2)
                    nc.vector.tensor_scalar(
                        dr[0:hs, :], dsum[0:hs, :], 0.0, None, ALU.max
                    )
                    dcol = AUX_W2A + (dt_ if dt_ < 2 else 2)
                    nc.tensor.matmul(
                        dps[0:1, :],
                        lhsT=aux_sb[0:hs, dcol:dcol + 1],
                        rhs=dr[0:hs, :],
                        start=(dt_ == 0), stop=(dt_ == HT - 1),
                    )
                nc.scalar.activation(dlog[0:1, :], dps[0:1, :], AF.Identity)
                nc.sync.dma_start(outD[0:1, :], dlog[0:1, :])

        # one output DMA for all 4 strips' accumulators; host decodes
        # S for local i from row 32c+q of col c.
        nc.sync.dma_start(outS[:, :], sexp4[:, :])

        for p in (pp_d, pp_z, pp_pre, rpool, cpool):
            p.release()

    nc.finalize()
    return nc


def _get_module():
    if "nc" not in _CACHE:
        _CACHE["nc"] = _build_module()
    return _CACHE["nc"]


def kernel(**inputs) -> np.ndarray:
    import ml_dtypes
    from concourse.bass_utils import run_bass_kernel_spmd

    bf = ml_dtypes.bfloat16
    x = np.ascontiguousarray(np.asarray(inputs["x_samples"], dtype=np.float32))
    y = np.ascontiguousarray(np.asarray(inputs["y_samples"], dtype=np.float32))
    W1 = np.asarray(inputs["W1"], dtype=np.float32)
    b1 = np.asarray(inputs["b1"], dtype=np.float32).reshape(H)
    W2 = np.asarray(inputs["W2"], dtype=np.float32)
    b2 = float(np.asarray(inputs["b2"], dtype=np.float32).reshape(1)[0])

    def sbuf_fmt(aT):
        # [KD*128, F] transposed matrix -> SBUF tile layout [128, KD*F]
        # (row p, col k*F+f = aT[128k+p, f]), contiguous.
        kd = aT.shape[0] // 128
        return np.ascontiguousarray(
            aT.reshape(kd, 128, -1).transpose(1, 0, 2).reshape(128, -1).astype(bf)
        )

    xt_p = sbuf_fmt(x.T)             # [128, 6*512]
    w1x_p = sbuf_fmt(W1[:, :XD].T)   # [128, 6*300]
    w1y_p = sbuf_fmt(W1[:, XD:].T)   # [128, 6*300]

    bcons = np.zeros((128, HT + 1), np.float32)
    w2 = W2.reshape(H)
    hs2 = HSZ[2]
    for t in range(2):
        bcons[:, t] = b1[128 * t:128 * (t + 1)]
    bcons[:hs2, 2] = b1[256:256 + hs2]
    bcons[:, 3] = b2

    # bf16 weight block: diag cols 0:4, then 40 16-col q-blocks (sec0 = t0
    # q 0..15, sec1 = t1, sec2 = t2 pairs m 0..7).
    w2a = np.zeros((128, W2A_W), np.float32)
    w2a[:, 0] = w2[0:128]
    w2a[:, 1] = w2[128:256]
    w2a[:hs2, 2] = w2[256:256 + hs2]
    for q in range(16):
        w2a[:, 4 + (0 + q) * 16 + q] = w2[0:128]
        w2a[:, 4 + (16 + q) * 16 + q] = w2[128:256]
    for m in range(8):
        w2a[:hs2, 4 + (32 + m) * 16 + 2 * m] = w2[256:256 + hs2]
        w2a[64:64 + hs2, 4 + (32 + m) * 16 + 2 * m + 1] = w2[256:256 + hs2]

    b1row = np.zeros((1, 3 * 128), np.float32)
    for t in range(2):
        b1row[0, 128 * t:128 * (t + 1)] = b1[128 * t:128 * (t + 1)]
    b1row[0, 256:256 + hs2] = b1[256:256 + hs2]

    in_maps = []
    for c in range(NCORES):
        sl = slice(c * ISH, (c + 1) * ISH)
        aux = np.concatenate(
            [sbuf_fmt(y[sl].T), sbuf_fmt(x[sl].T), w2a.astype(bf)], axis=1
        )
        in_maps.append({
            "xt_p": xt_p,
            "w1x_p": w1x_p,
            "w1y_p": w1y_p,
            "aux_p": np.ascontiguousarray(aux),
            "bcons": bcons,
            "b1row": np.ascontiguousarray(b1row.astype(bf)),
        })

    nc = _get_module()
    res = run_bass_kernel_spmd(
        nc, in_maps, core_ids=list(range(NCORES)), trace=TRACE
    )
    global LAST_RESULTS
    LAST_RESULTS = res

    # device outS[:, c]: S for local i at row 32c+q of col c, with
    # c = (i//2)%4, q = 2*(i//8) + i%2.
    ii = np.arange(ISH)
    cc = (ii // 2) % 4
    qq = 2 * (ii // 8) + (ii % 2)
    S = np.concatenate(
        [r["outS"].reshape(128, 4)[32 * cc + qq, cc] for r in res.results]
    ).astype(np.float64)
    d = np.concatenate([r["outD"].reshape(ISH) for r in res.results]).astype(np.float64)
    v = d + b2
    t0 = np.log1p(np.exp(-np.abs(v))) + np.maximum(v, 0.0)   # softplus(diag + b2)
    lse = np.log(float(N) + S)
    val = t0.mean() - (lse.mean() - math.log(N))
    return np.float32(val)
